# revision 8
# baseline (speedup 1.0000x reference)
"""Multi-head attention (B=1, S=4096, E=768, H=12, Dk=64) on 8 TRN2 NeuronCores.

Sharding: 4 head-groups (3 heads) x 2 seq-halves (2048 queries). Each core:
  - projects K^T, V for its 3 heads over the FULL (rolled) sequence,
    Q^T over its 2048 queries, from a replicated x^T input (bf16 matmuls)
  - computes scores^T = K Q^T per 128-key chunk (keys on partitions).
    Even key chunks use bf16 matmuls; odd chunks use fp8e4 DoubleRow
    matmuls (2 zero-padded k-tiles, 0.5 cycles/row) for ~2x PE throughput
    at a small accuracy cost (mixing keeps total error in budget).
  - exp via ACT (exact, fp8e4 out) for most groups; a tunable share runs
    on DVE as a fast-exp bit trick (t = s*log2e*8 + 55.54 -> int8 ->
    bitcast fp8e4; constant calibrated against HW round-to-nearest so the
    multiplicative bias is ~0)
  - P @ [V8 | 1] and P @ Vr8 (fp8 DoubleRow over key-chunk pairs) give the
    attention output, V residual compensation (V = V8 + Vr8) restoring
    ~bf16 V accuracy; softmax denominators come from the ones column.
    Accumulated into SBUF; normalized with DVE reciprocal + gpsimd mult.
  - transposes the normalized output and applies the local slice of the
    out-projection (bias folded in via a ones-row), giving a partial
    [2048, 768] result
Host: sums the 4 head-group partials per seq-half, concatenates halves.

PSUM notes: start_tensor_calc zeroes the WHOLE 2KiB bank; DoubleRow matmul
PSUM outputs must be 512B-aligned within the bank (HW constraint, found
empirically) -> po accumulator uses 128-wide slots.

Self-contained: shapes/sharding hardcoded.
"""
import numpy as np
import ml_dtypes

import concourse.bass as bass
import concourse.mybir as mybir
import concourse.tile as tile
from concourse import bacc
from concourse.bass_utils import run_bass_kernel_spmd
from concourse.masks import make_identity

# Problem constants
S = 4096          # sequence length
S2 = S // 2       # odd-chunk fp8 K storage
E = 768           # embed dim
H = 12            # heads
DK = 64           # head dim
HPC = 3           # heads per core
DL = HPC * DK     # local head dims = 192
QL = S // 2       # local queries = 2048
N_CORES = 8
ECH = E // 128    # 6 e-chunks
ST = S // 512     # 8 seq tiles of 512
QST = QL // 512   # 4 query seq tiles
KC = S // 128     # 32 key chunks of 128
KC2 = KC // 2
F32R = mybir.dt.float32r
F32 = mybir.dt.float32
BF16 = mybir.dt.bfloat16
F8 = mybir.dt.float8e4   # e4m3 (ml_dtypes.float8_e4m3, IEEE, bias 7)
I8 = mybir.dt.int8
DR = mybir.MatmulPerfMode.DoubleRow
MUL = mybir.AluOpType.mult
ADD = mybir.AluOpType.add

# fast-exp bit trick: fp8e4 bits = s*0.125*log2(e)*8 + B  (B calibrated on
# HW: DVE f32->int8 rounds to nearest; B centers the multiplicative bias)
EXP_A = 1.442695
EXP_B = 55.54

_NC_CACHE = {}

OPTS = {"proj_tag": "sm", "pts_bufs": 4, "interleave": True,
        "piece_order": "spread", "xts_bufs": 2, "const_gpsimd": True,
        "tr_tag": "sm", "kcg": 2, "sm_bufs": 2, "big_bufs": 3,
        "cap_kc": 16, "tail_pops": 2, "warmup": 10, "small_bufs": 8,
        "outs_bufs": 6, "unit_order": "default",
        "qk8": 1,        # fp8 DoubleRow scores on odd key chunks
        "dve_g": 64,     # of the 192 exp groups, how many go to DVE
        }


def build_nc(**opts):
    """Build and compile the SPMD per-core program (same NEFF for all cores)."""
    o = dict(OPTS); o.update(opts)
    key = tuple(sorted(o.items()))
    if key in _NC_CACHE:
        return _NC_CACHE[key]
    nc = bacc.Bacc("TRN2", target_bir_lowering=False, debug=False,
                   num_devices=N_CORES)
    xt = nc.dram_tensor("xt", [E, S], BF16, kind="ExternalInput")
    wqk = nc.dram_tensor("wqk", [E, 2 * DL], BF16, kind="ExternalInput")
    wv = nc.dram_tensor("wv", [E, 192], BF16, kind="ExternalInput")
    wo = nc.dram_tensor("wo", [DL + 1, E], BF16, kind="ExternalInput")
    onesrow = nc.dram_tensor("onesrow", [1, QL], BF16, kind="ExternalInput")
    out = nc.dram_tensor("out", [QL, E], F32, kind="ExternalOutput")

    with tile.TileContext(nc) as tc:
        with (
            tc.tile_pool(name="consts", bufs=1) as consts,
            tc.tile_pool(name="kv", bufs=1) as kv,
            tc.tile_pool(name="xts", bufs=o["xts_bufs"]) as xts,
            tc.tile_pool(name="pts", bufs=o["pts_bufs"]) as pts,
            tc.tile_pool(name="small", bufs=o["small_bufs"]) as small,
            tc.tile_pool(name="outs", bufs=o["outs_bufs"]) as outs,
            tc.tile_pool(name="psum", bufs=o["big_bufs"], space="PSUM") as psum,
            tc.tile_pool(name="psm", bufs=o["sm_bufs"], space="PSUM") as psm,
        ):
            KCGo = o["kcg"]
            QK8 = o["qk8"]
            # ---- resident constants ----
            cdma = nc.gpsimd if o["const_gpsimd"] else nc.sync
            wqk_sb = consts.tile([128, ECH, 2 * DL], BF16)
            cdma.dma_start(out=wqk_sb, in_=wqk.rearrange("(c p) n -> p c n", p=128))
            wv_sb = consts.tile([128, ECH, 192], BF16)
            cdma.dma_start(out=wv_sb, in_=wv.rearrange("(c p) n -> p c n", p=128))
            woA = consts.tile([128, E], BF16)
            cdma.dma_start(out=woA, in_=wo[0:128, :])
            woB = consts.tile([65, E], BF16)
            cdma.dma_start(out=woB, in_=wo[128:193, :])
            ident = consts.tile([128, 128], BF16)
            make_identity(nc, ident)
            zcol = consts.tile([1, 128], BF16)
            nc.vector.memset(zcol, 0.0)
            zrow = consts.tile([1, 512], BF16)
            nc.vector.memset(zrow, 0.0)

            # ---- resident K^T / Q^T / V / accumulators ----
            ktA = kv.tile([128, S], BF16)    # K^T dims 0-127 (h0, h1)
            ktB = kv.tile([64, S], BF16)     # K^T dims 128-191 (h2)
            qtA = kv.tile([128, QL], BF16)   # Q^T dims 0-127
            qtB = kv.tile([64, QL], BF16)    # Q^T dims 128-191
            if QK8:
                # odd key chunks, fp8: [dims, k-tile, cols]; tile1 zeros
                kt8A = kv.tile([128, 2, S2], F8)
                kt8B = kv.tile([64, 2, S2], F8)
                qt8A = kv.tile([128, 2, QL], F8)
                qt8B = kv.tile([64, 2, QL], F8)
                for t in (kt8A, kt8B, qt8A, qt8B):
                    nc.gpsimd.memset(t[:, 1, :], 0.0)
            v_sb = kv.tile([128, KC, HPC, 65], F8)   # V8 natural + ones col
            nc.vector.memset(v_sb[:, :, :, 64:65], 1.0)
            vr_sb = kv.tile([128, KC, HPC, 64], F8)  # V residual (V - V8)
            po_acc = kv.tile([128, HPC * QST, QST, 65], F32)  # PV partials

            # PE warm-up: ~3us of dummy matmuls on const zeros while the
            # first x^T DMAs land, so the HAM clock-gate opens (1.2->2.4GHz)
            # before the real projection matmuls issue.
            if o["warmup"]:
                pwu = psm.tile([128, 512], F32, tag="sm")
                for i in range(o["warmup"]):
                    nc.tensor.matmul(pwu, zcol, zrow, start=True, stop=True,
                                     skip_group_check=True)

            atA = kv.tile([128, QL], BF16)   # A^T dims 0-127
            atB = kv.tile([65, QL], BF16)    # A^T dims 128-191 + ones row
            nc.sync.dma_start(out=atB[64:65, :], in_=onesrow[:, :])

            def kt_h(h, kc):
                if h < 2:
                    return ktA[h * 64:(h + 1) * 64, kc * 128:(kc + 1) * 128]
                return ktB[0:64, kc * 128:(kc + 1) * 128]

            def qt_h(h, qb):
                if h < 2:
                    return qtA[h * 64:(h + 1) * 64, qb * 512:(qb + 1) * 512]
                return qtB[0:64, qb * 512:(qb + 1) * 512]

            def kt8_h(h, kc):
                c = kc // 2
                if h < 2:
                    return kt8A[h * 64:(h + 1) * 64, :, c * 128:(c + 1) * 128]
                return kt8B[0:64, :, c * 128:(c + 1) * 128]

            def qt8_h(h, qb):
                if h < 2:
                    return qt8A[h * 64:(h + 1) * 64, :, qb * 512:(qb + 1) * 512]
                return qt8B[0:64, :, qb * 512:(qb + 1) * 512]

            def proj_psum():
                if o["proj_tag"] == "sm":
                    py = psm.tile([128, 512], F32, tag="sm", name="py")
                    return py
                py3 = psum.tile([128, KCGo, 512], F32, tag="big")
                return py3[:, 0, :]

            def phase1_pieces(st):
                """Returns a list of closures: [dma, rc..., vp x4]."""
                xt_box = []

                def dma():
                    xt_sb = xts.tile([128, ECH, 512], BF16, tag="xt")
                    for ec in range(ECH):
                        nc.sync.dma_start(
                            out=xt_sb[:, ec, :],
                            in_=xt[ec * 128:(ec + 1) * 128,
                                   st * 512:(st + 1) * 512],
                        )
                    xt_box.append(xt_sb)

                def k_copies(py_half, base, ktX, kt8X):
                    """py rows -> bf16 kt cols (all chunks) + fp8 odd chunks."""
                    c0, c1 = st * 512, (st + 1) * 512
                    nc.vector.tensor_copy(ktX[base:base + 64, c0:c1], py_half)
                    if QK8:
                        py4 = py_half.rearrange("p (a b) -> p a b", b=128)
                        d0 = st * 256
                        nc.vector.tensor_copy(
                            kt8X[base:base + 64, 0, d0:d0 + 256].rearrange(
                                "p (a b) -> p a b", b=128),
                            py4[:, 1:4:2, :])

                def q_copies(py_half, base, qtX, qt8X):
                    c0, c1 = st * 512, (st + 1) * 512
                    nc.vector.tensor_copy(qtX[base:base + 64, c0:c1], py_half)
                    if QK8:
                        nc.vector.tensor_copy(
                            qt8X[base:base + 64, 0, c0:c1], py_half)

                def qk(rc):
                    xt_sb = xt_box[0]
                    py = proj_psum()
                    for ec in range(ECH):
                        nc.tensor.matmul(
                            py,
                            wqk_sb[:, ec, rc * 128:(rc + 1) * 128],
                            xt_sb[:, ec, :],
                            start=(ec == 0), stop=(ec == ECH - 1),
                        )
                    if rc == 0:
                        q_copies(py[0:64, :], 0, qtA, qt8A if QK8 else None)
                        q_copies(py[64:128, :], 64, qtA, qt8A if QK8 else None)
                    elif rc == 1:
                        if st < QST:
                            q_copies(py[0:64, :], 0, qtB, qt8B if QK8 else None)
                        k_copies(py[64:128, :], 0, ktA, kt8A if QK8 else None)
                    else:
                        k_copies(py[0:64, :], 64, ktA, kt8A if QK8 else None)
                        k_copies(py[64:128, :], 0, ktB, kt8B if QK8 else None)

                def vp(j):
                    xt_sb = xt_box[0]
                    pv = proj_psum()[:, 0:192]
                    for ec in range(ECH):
                        nc.tensor.matmul(
                            pv,
                            xt_sb[:, ec, j * 128:(j + 1) * 128],
                            wv_sb[:, ec, :],
                            start=(ec == 0), stop=(ec == ECH - 1),
                        )
                    kc = st * 4 + j
                    pv3 = pv.rearrange("p (h d) -> p h d", d=64)
                    v8 = v_sb[:, kc, :, 0:64]
                    nc.vector.tensor_copy(v8, pv3)
                    nc.vector.tensor_sub(vr_sb[:, kc, :, :], pv3, v8)

                rcs = (0, 1, 2) if st < QST else (1, 2)
                pieces = [dma]
                pieces += [(lambda rc=rc: qk(rc)) for rc in rcs]
                pieces += [(lambda j=j: vp(j)) for j in range(4)]
                return pieces

            def phase1(st):
                for p in phase1_pieces(st):
                    p()

            # attention unit: one (key-group g, head h, query-block qb)
            kcgs = [(g * KCGo, min(KCGo, KC - g * KCGo))
                    for g in range((KC + KCGo - 1) // KCGo)]

            started_pairs = set()
            exp_ctr = [0]
            n_groups_total = len(kcgs) * HPC * QST

            def exp_engine():
                i = exp_ctr[0]
                exp_ctr[0] += 1
                f = o["dve_g"]
                return "D" if (i * f) // n_groups_total != \
                    ((i + 1) * f) // n_groups_total else "A"

            def attn_unit(gs, h, qb):
                """Several key-groups of one (h, qb) into one PSUM
                accumulator (128-wide slots for DoubleRow alignment)."""
                po = psm.tile([128, QST, 128], F32, tag="sm")
                nc.tensor.matmul(po[:, 0, 0:1], zcol, zrow[:, 0:1],
                                 start=True, stop=False,
                                 skip_group_check=True)
                for gi, g in enumerate(gs):
                    kc0, klen = kcgs[g]
                    ps = psum.tile([128, KCGo, 512], F32, tag="big")
                    for j in range(klen):
                        kc = kc0 + j
                        if QK8 and (kc % 2 == 1):
                            nc.tensor.matmul(
                                ps[:, j, :], kt8_h(h, kc), qt8_h(h, qb),
                                start=True, stop=True, perf_mode=DR,
                            )
                        else:
                            nc.tensor.matmul(
                                ps[:, j, :], kt_h(h, kc), qt_h(h, qb),
                                start=True, stop=True,
                            )
                    pt = pts.tile([128, KCGo, 512], F8, tag="pt")
                    if exp_engine() == "A":
                        nc.scalar.activation(
                            pt[:, 0:klen, :], ps[:, 0:klen, :],
                            mybir.ActivationFunctionType.Exp, scale=0.125,
                        )
                    else:
                        nc.vector.tensor_scalar(
                            pt[:, 0:klen, :].bitcast(I8), ps[:, 0:klen, :],
                            EXP_A, EXP_B, op0=MUL, op1=ADD,
                        )
                    last_u = gi == len(gs) - 1
                    # P@[V8|1] + P@Vr8 as fp8 DoubleRow over kc pairs
                    for qt in range(QST):
                        qc = qt * 128
                        for jj in range(0, klen, 2):
                            last = (last_u and qt == QST - 1 and
                                    jj == klen - 2)
                            nc.tensor.matmul(
                                po[:, qt, 0:65],
                                pt[:, jj:jj + 2, qc:qc + 128],
                                v_sb[:, kc0 + jj:kc0 + jj + 2, h, :],
                                start=False, stop=False,
                                perf_mode=DR, skip_group_check=True,
                            )
                            nc.tensor.matmul(
                                po[:, qt, 0:64],
                                pt[:, jj:jj + 2, qc:qc + 128],
                                vr_sb[:, kc0 + jj:kc0 + jj + 2, h, :],
                                start=False, stop=last,
                                perf_mode=DR, skip_group_check=True,
                            )
                acc = po_acc[:, h * QST + qb, :, :]
                pov = po[:, :, 0:65]
                if (h, qb) not in started_pairs:
                    started_pairs.add((h, qb))
                    nc.vector.tensor_copy(acc, pov)
                else:
                    nc.vector.tensor_add(acc, acc, pov)

            def normalize(h, qb):
                for qt in range(QST):
                    acc = po_acc[:, h * QST + qb, qt, :]
                    rec = small.tile([128, 1], F32, tag="rec")
                    nc.vector.reciprocal(rec, acc[:, 64:65])
                    a_sb = small.tile([128, 64], BF16, tag="a")
                    nc.gpsimd.tensor_scalar(
                        a_sb, acc[:, 0:64], rec, None, op0=MUL)
                    ptr = (psm if o["tr_tag"] == "sm" else psum).tile(
                        [64, 128], BF16, tag=o["tr_tag"])
                    nc.tensor.transpose(ptr, a_sb, ident)
                    qcol = qb * 512 + qt * 128
                    if h < 2:
                        nc.vector.tensor_copy(
                            atA[h * 64:(h + 1) * 64, qcol:qcol + 128], ptr)
                    else:
                        nc.vector.tensor_copy(
                            atB[0:64, qcol:qcol + 128], ptr)

            def _op_half(q, pf, n0, n1):
                nc.tensor.matmul(pf[:, n0:n1],
                                 atA[:, q * 128:(q + 1) * 128],
                                 woA[:, n0:n1], start=True, stop=False)
                nc.tensor.matmul(pf[:, n0:n1],
                                 atB[:, q * 128:(q + 1) * 128],
                                 woB[:, n0:n1], start=False, stop=True)

            def outproj_q(q):
                # needs atA/atB complete for all heads at these query columns
                pf3 = psum.tile([128, KCGo, 512], F32, tag="big")
                pf = pf3.rearrange("p a b -> p (a b)")
                _op_half(q, pf, 0, 512)
                _op_half(q, pf, 512, 768)
                ob = outs.tile([128, E], F32, tag="ob")
                nc.vector.tensor_copy(ob, pf[:, 0:E])
                nc.sync.dma_start(out=out[q * 128:(q + 1) * 128, :], in_=ob)

            # ---- software-pipelined emission ----
            # unit (g, h, qb) is ready once seq-tile max(st(g), qb) is
            # projected; groups of one (h, qb) landing in the same slot are
            # merged (up to cap) to share an accumulator bank + DVE add.
            raw_at = {s: [] for s in range(ST)}
            for g, (kc0, klen) in enumerate(kcgs):
                st_g = (kc0 + klen - 1) // 4
                for qb in range(QST):
                    for h in range(HPC):
                        raw_at[max(st_g, qb)].append((g, h, qb))
            units_at = {}
            for s in range(ST):
                merged = {}
                order = []
                for (g, h, qb) in raw_at[s]:
                    if (h, qb) not in merged:
                        merged[(h, qb)] = []
                        order.append((h, qb))
                    merged[(h, qb)].append(g)
                units = []
                cap = max(1, o["cap_kc"] // KCGo)
                for (h, qb) in order:
                    gs = sorted(merged[(h, qb)])
                    for i in range(0, len(gs), cap):
                        units.append((gs[i:i + cap], h, qb))
                units_at[s] = units
            if o["unit_order"] == "qb":
                for s in range(ST):
                    units_at[s].sort(key=lambda u: (u[2], u[1]))
            elif o["unit_order"] == "h":
                for s in range(ST):
                    units_at[s].sort(key=lambda u: (u[1], u[2]))
            last_s = ST - 1
            # final batch: qb-major so each qb's normalize+outproj fuses in
            units_at[last_s].sort(key=lambda u: (u[2], u[1]))

            def emit_batch(s, units):
                """Interleave next seq-tile's projection pieces among units."""
                if o["interleave"] and s + 1 < ST:
                    pieces = phase1_pieces(s + 1)
                else:
                    pieces = []
                work = []
                if o["piece_order"] == "front":
                    work += [("p", p) for p in pieces]
                    work += [("u", u) for u in units]
                else:
                    n = max(len(units), 1)
                    per = len(pieces) / n
                    acc_p = 0.0
                    pi = 0
                    for i, u in enumerate(units):
                        work.append(("u", u))
                        acc_p += per
                        while pi < len(pieces) and acc_p >= pi + 1:
                            work.append(("p", pieces[pi]))
                            pi += 1
                    while pi < len(pieces):
                        work.append(("p", pieces[pi]))
                        pi += 1
                # In the last batch, a qb's normalize/outproj tail is deferred
                # and interleaved with the NEXT qb's units, so ACT keeps
                # running exps while PE does the tail matmuls.
                remaining = {}
                if s == last_s:
                    for (gs, h, qb) in units:
                        remaining[qb] = remaining.get(qb, 0) + 1
                pending_tail = []

                def tail_pieces(qb):
                    ps = [(lambda hh=hh, qb=qb: normalize(hh, qb))
                          for hh in range(HPC)]
                    ps += [(lambda qq=qq, qb=qb: outproj_q(qb * 4 + qq))
                           for qq in range(4)]
                    return ps

                for kind, item in work:
                    if kind == "u":
                        gs, h, qb = item
                        attn_unit(gs, h, qb)
                        for _ in range(o["tail_pops"]):
                            if pending_tail:
                                pending_tail.pop(0)()
                        if s == last_s:
                            remaining[qb] -= 1
                            if remaining[qb] == 0:
                                pending_tail += tail_pieces(qb)
                    else:
                        item()
                for p in pending_tail:
                    p()

            if o["interleave"]:
                phase1(0)
                for s in range(ST):
                    emit_batch(s, units_at[s])
            else:
                for s in range(ST):
                    phase1(s)
                for s in range(ST):
                    emit_batch(s, units_at[s])

    nc.compile()
    _NC_CACHE[key] = nc
    return nc


def make_in_maps(x, w_qkv, w_out, b_out):
    """Shard full inputs into 8 per-core input maps."""
    x = np.asarray(x, dtype=np.float32)
    w_qkv = np.asarray(w_qkv, dtype=np.float32)
    w_out = np.asarray(w_out, dtype=np.float32)
    b_out = np.asarray(b_out, dtype=np.float32)
    xt = np.ascontiguousarray(x.reshape(S, E).T).astype(
        ml_dtypes.bfloat16)                               # [E, S]
    xt_roll = np.ascontiguousarray(
        np.concatenate([xt[:, QL:], xt[:, :QL]], axis=1))  # for seq-half 1
    in_maps = []
    for c in range(N_CORES):
        hg, sh = c // 2, c % 2
        q_rows = w_qkv[hg * DL:(hg + 1) * DL]             # [192, 768]
        k_rows = w_qkv[E + hg * DL:E + (hg + 1) * DL]
        v_rows = w_qkv[2 * E + hg * DL:2 * E + (hg + 1) * DL]
        wqk_in = np.ascontiguousarray(
            np.concatenate([q_rows, k_rows], axis=0).T).astype(
                ml_dtypes.bfloat16)                       # [768, 384]
        wv_in = np.ascontiguousarray(v_rows.T).astype(ml_dtypes.bfloat16)
        wo_in = np.zeros((DL + 1, E), np.float32)
        wo_in[:DL] = w_out[:, hg * DL:(hg + 1) * DL].T    # [192, 768]
        if hg == 0:
            wo_in[DL] = b_out
        in_maps.append({
            "onesrow": np.ones((1, QL), ml_dtypes.bfloat16),
            "xt": xt if sh == 0 else xt_roll,
            "wqk": wqk_in,
            "wv": wv_in,
            "wo": np.ascontiguousarray(wo_in).astype(ml_dtypes.bfloat16),
        })
    return in_maps


def gather_out(results):
    """Sum head-group partials per seq-half, concat halves -> [1, S, E]."""
    halves = []
    for sh in range(2):
        acc = np.zeros((QL, E), np.float64)
        for hg in range(4):
            acc += results[hg * 2 + sh]["out"]
        halves.append(acc.astype(np.float32))
    return np.concatenate(halves, axis=0)[None]


def kernel(x, w_qkv, w_out, b_out):
    nc = build_nc()
    in_maps = make_in_maps(x, w_qkv, w_out, b_out)
    res = run_bass_kernel_spmd(nc, in_maps, core_ids=list(range(N_CORES)))
    return gather_out(res.results)


# revision 13
# speedup vs baseline: 1.0818x; 1.0818x over previous
"""Multi-head attention (B=1, S=4096, E=768, H=12, Dk=64) on 8 TRN2 NeuronCores.

Sharding: 4 head-groups (3 heads) x 2 seq-halves (2048 queries). Each core:
  - projects K^T, V for its 3 heads over the FULL (rolled) sequence,
    Q^T over its 2048 queries, from a replicated x^T input (bf16 matmuls)
  - computes scores^T = K Q^T per 128-key chunk (keys on partitions).
    Even key chunks use bf16 matmuls; odd chunks use fp8e4 DoubleRow
    matmuls (2 zero-padded k-tiles, 0.5 cycles/row) for ~2x PE throughput
    at a small accuracy cost (mixing keeps total error in budget).
  - exp via ACT (exact, fp8e4 out) for most groups; a tunable share runs
    on DVE as a fast-exp bit trick (t = s*log2e*8 + 55.54 -> int8 ->
    bitcast fp8e4; constant calibrated against HW round-to-nearest so the
    multiplicative bias is ~0)
  - P @ [V8 | 1] and P @ Vr8 (fp8 DoubleRow over key-chunk pairs) give the
    attention output, V residual compensation (V = V8 + Vr8) restoring
    ~bf16 V accuracy; softmax denominators come from the ones column.
    Accumulated into SBUF; normalized with DVE reciprocal + gpsimd mult.
  - transposes the normalized output and applies the local slice of the
    out-projection (bias folded in via a ones-row), giving a partial
    [2048, 768] result
Host: sums the 4 head-group partials per seq-half, concatenates halves.

PSUM notes: start_tensor_calc zeroes the WHOLE 2KiB bank; DoubleRow matmul
PSUM outputs must be 512B-aligned within the bank (HW constraint, found
empirically) -> po accumulator uses 128-wide slots.

Self-contained: shapes/sharding hardcoded.
"""
import numpy as np
import ml_dtypes

import concourse.bass as bass
import concourse.mybir as mybir
import concourse.tile as tile
from concourse import bacc
from concourse.bass_utils import run_bass_kernel_spmd
from concourse.masks import make_identity

# Problem constants
S = 4096          # sequence length
S2 = S // 2       # odd-chunk fp8 K storage
E = 768           # embed dim
H = 12            # heads
DK = 64           # head dim
HPC = 3           # heads per core
DL = HPC * DK     # local head dims = 192
QL = S // 2       # local queries = 2048
N_CORES = 8
ECH = E // 128    # 6 e-chunks
ST = S // 512     # 8 seq tiles of 512
QST = QL // 512   # 4 query seq tiles
KC = S // 128     # 32 key chunks of 128
KC2 = KC // 2
F32R = mybir.dt.float32r
F32 = mybir.dt.float32
BF16 = mybir.dt.bfloat16
F8 = mybir.dt.float8e4   # e4m3 (ml_dtypes.float8_e4m3, IEEE, bias 7)
I8 = mybir.dt.int8
DR = mybir.MatmulPerfMode.DoubleRow
MUL = mybir.AluOpType.mult
ADD = mybir.AluOpType.add

# fast-exp bit trick: fp8e4 bits = s*0.125*log2(e)*8 + B  (B calibrated on
# HW: DVE f32->int8 rounds to nearest; B centers the multiplicative bias)
EXP_A = 1.442695
EXP_B = 55.54

_NC_CACHE = {}

OPTS = {"proj_tag": "sm", "pts_bufs": 4, "interleave": True,
        "piece_order": "spread", "xts_bufs": 2, "const_gpsimd": True,
        "tr_tag": "sm", "kcg": 2, "sm_bufs": 2, "big_bufs": 3,
        "cap_kc": 16, "tail_pops": 2, "warmup": 10, "small_bufs": 8,
        "outs_bufs": 6, "unit_order": "default",
        "qk8": 1,        # fp8 DoubleRow scores on odd key chunks
        "dve_g": 92,     # of the 192 exp groups, how many go to DVE
        "at_eng": "A",   # engine for at-copies: A=ACT, D=DVE
        "cp8_eng": "A",  # engine for q8/k8/v8 convert copies
        "slot2": 1,      # round unit slots up to odd (halves unit count)
        "vp2": 1,        # project V two key-chunks per piece
        }


def build_nc(**opts):
    """Build and compile the SPMD per-core program (same NEFF for all cores)."""
    o = dict(OPTS); o.update(opts)
    key = tuple(sorted(o.items()))
    if key in _NC_CACHE:
        return _NC_CACHE[key]
    nc = bacc.Bacc("TRN2", target_bir_lowering=False, debug=False,
                   num_devices=N_CORES)
    xt = nc.dram_tensor("xt", [E, S], BF16, kind="ExternalInput")
    wqk = nc.dram_tensor("wqk", [E, 2 * DL], BF16, kind="ExternalInput")
    wv = nc.dram_tensor("wv", [E, 192], BF16, kind="ExternalInput")
    wo = nc.dram_tensor("wo", [DL + 1, E], BF16, kind="ExternalInput")
    onesrow = nc.dram_tensor("onesrow", [1, QL], BF16, kind="ExternalInput")
    out = nc.dram_tensor("out", [QL, E], F32, kind="ExternalOutput")

    with tile.TileContext(nc) as tc:
        with (
            tc.tile_pool(name="consts", bufs=1) as consts,
            tc.tile_pool(name="kv", bufs=1) as kv,
            tc.tile_pool(name="xts", bufs=o["xts_bufs"]) as xts,
            tc.tile_pool(name="pts", bufs=o["pts_bufs"]) as pts,
            tc.tile_pool(name="small", bufs=o["small_bufs"]) as small,
            tc.tile_pool(name="outs", bufs=o["outs_bufs"]) as outs,
            tc.tile_pool(name="psum", bufs=o["big_bufs"], space="PSUM") as psum,
            tc.tile_pool(name="psm", bufs=o["sm_bufs"], space="PSUM") as psm,
        ):
            KCGo = o["kcg"]
            QK8 = o["qk8"]
            # ---- resident constants ----
            cdma = nc.gpsimd if o["const_gpsimd"] else nc.sync
            wqk_sb = consts.tile([128, ECH, 2 * DL], BF16)
            cdma.dma_start(out=wqk_sb, in_=wqk.rearrange("(c p) n -> p c n", p=128))
            wv_sb = consts.tile([128, ECH, 192], BF16)
            cdma.dma_start(out=wv_sb, in_=wv.rearrange("(c p) n -> p c n", p=128))
            woA = consts.tile([128, E], BF16)
            cdma.dma_start(out=woA, in_=wo[0:128, :])
            woB = consts.tile([65, E], BF16)
            cdma.dma_start(out=woB, in_=wo[128:193, :])
            ident = consts.tile([128, 128], BF16)
            make_identity(nc, ident)
            zcol = consts.tile([1, 128], BF16)
            nc.vector.memset(zcol, 0.0)
            zrow = consts.tile([1, 512], BF16)
            nc.vector.memset(zrow, 0.0)

            # ---- resident K^T / Q^T / V / accumulators ----
            ktA = kv.tile([128, S], BF16)    # K^T dims 0-127 (h0, h1)
            ktB = kv.tile([64, S], BF16)     # K^T dims 128-191 (h2)
            qtA = kv.tile([128, QL], BF16)   # Q^T dims 0-127
            qtB = kv.tile([64, QL], BF16)    # Q^T dims 128-191
            if QK8:
                # odd key chunks, fp8: [dims, k-tile, cols]; tile1 zeros
                kt8A = kv.tile([128, 2, S2], F8)
                kt8B = kv.tile([64, 2, S2], F8)
                qt8A = kv.tile([128, 2, QL], F8)
                qt8B = kv.tile([64, 2, QL], F8)
                for t in (kt8A, kt8B, qt8A, qt8B):
                    nc.gpsimd.memset(t[:, 1, :], 0.0)
            v_sb = kv.tile([128, KC, HPC, 65], F8)   # V8 natural + ones col
            nc.vector.memset(v_sb[:, :, :, 64:65], 1.0)
            vr_sb = kv.tile([128, KC, HPC, 64], F8)  # V residual (V - V8)
            po_acc = kv.tile([128, HPC * QST, QST, 65], F32)  # PV partials

            # PE warm-up: ~3us of dummy matmuls on const zeros while the
            # first x^T DMAs land, so the HAM clock-gate opens (1.2->2.4GHz)
            # before the real projection matmuls issue.
            if o["warmup"]:
                pwu = psm.tile([128, 512], F32, tag="sm")
                for i in range(o["warmup"]):
                    nc.tensor.matmul(pwu, zcol, zrow, start=True, stop=True,
                                     skip_group_check=True)

            atA = kv.tile([128, QL], BF16)   # A^T dims 0-127
            atB = kv.tile([65, QL], BF16)    # A^T dims 128-191 + ones row
            nc.sync.dma_start(out=atB[64:65, :], in_=onesrow[:, :])

            def kt_h(h, kc):
                if h < 2:
                    return ktA[h * 64:(h + 1) * 64, kc * 128:(kc + 1) * 128]
                return ktB[0:64, kc * 128:(kc + 1) * 128]

            def qt_h(h, qb):
                if h < 2:
                    return qtA[h * 64:(h + 1) * 64, qb * 512:(qb + 1) * 512]
                return qtB[0:64, qb * 512:(qb + 1) * 512]

            def kt8_h(h, kc):
                c = kc // 2
                if h < 2:
                    return kt8A[h * 64:(h + 1) * 64, :, c * 128:(c + 1) * 128]
                return kt8B[0:64, :, c * 128:(c + 1) * 128]

            def qt8_h(h, qb):
                if h < 2:
                    return qt8A[h * 64:(h + 1) * 64, :, qb * 512:(qb + 1) * 512]
                return qt8B[0:64, :, qb * 512:(qb + 1) * 512]

            def proj_psum():
                if o["proj_tag"] == "sm":
                    py = psm.tile([128, 512], F32, tag="sm", name="py")
                    return py
                py3 = psum.tile([128, KCGo, 512], F32, tag="big")
                return py3[:, 0, :]

            def phase1_pieces(st):
                """Returns a list of closures: [dma, rc..., vp x4]."""
                xt_box = []

                def dma():
                    xt_sb = xts.tile([128, ECH, 512], BF16, tag="xt")
                    for ec in range(ECH):
                        nc.sync.dma_start(
                            out=xt_sb[:, ec, :],
                            in_=xt[ec * 128:(ec + 1) * 128,
                                   st * 512:(st + 1) * 512],
                        )
                    xt_box.append(xt_sb)

                def cp8(out_ap, in_ap):
                    """fp8 convert-copy on the knob-selected engine."""
                    if o["cp8_eng"] == "A":
                        nc.scalar.copy(out_ap, in_ap)
                    else:
                        nc.vector.tensor_copy(out_ap, in_ap)

                def k_copies(py_half, base, ktX, kt8X):
                    """py rows -> bf16 kt cols (all chunks) + fp8 odd chunks."""
                    c0, c1 = st * 512, (st + 1) * 512
                    nc.vector.tensor_copy(ktX[base:base + 64, c0:c1], py_half)
                    if QK8:
                        py4 = py_half.rearrange("p (a b) -> p a b", b=128)
                        d0 = st * 256
                        cp8(kt8X[base:base + 64, 0, d0:d0 + 256].rearrange(
                            "p (a b) -> p a b", b=128),
                            py4[:, 1:4:2, :])

                def qk(rc):
                    xt_sb = xt_box[0]
                    py = proj_psum()
                    for ec in range(ECH):
                        nc.tensor.matmul(
                            py,
                            wqk_sb[:, ec, rc * 128:(rc + 1) * 128],
                            xt_sb[:, ec, :],
                            start=(ec == 0), stop=(ec == ECH - 1),
                        )
                    c0, c1 = st * 512, (st + 1) * 512
                    if rc == 0:
                        nc.vector.tensor_copy(qtA[:, c0:c1], py)
                        if QK8:
                            cp8(qt8A[:, 0, c0:c1], py)
                    elif rc == 1:
                        if st < QST:
                            nc.vector.tensor_copy(qtB[0:64, c0:c1],
                                                  py[0:64, :])
                            if QK8:
                                cp8(qt8B[0:64, 0, c0:c1], py[0:64, :])
                        k_copies(py[64:128, :], 0, ktA, kt8A if QK8 else None)
                    else:
                        k_copies(py[0:64, :], 64, ktA, kt8A if QK8 else None)
                        k_copies(py[64:128, :], 0, ktB, kt8B if QK8 else None)

                def vp(j):
                    xt_sb = xt_box[0]
                    pv = proj_psum()[:, 0:192]
                    for ec in range(ECH):
                        nc.tensor.matmul(
                            pv,
                            xt_sb[:, ec, j * 128:(j + 1) * 128],
                            wv_sb[:, ec, :],
                            start=(ec == 0), stop=(ec == ECH - 1),
                        )
                    kc = st * 4 + j
                    pv3 = pv.rearrange("p (h d) -> p h d", d=64)
                    v8 = v_sb[:, kc, :, 0:64]
                    cp8(v8, pv3)
                    nc.vector.tensor_sub(vr_sb[:, kc, :, :], pv3, v8)

                def vp2(jp):
                    """Two key-chunks per piece: one PSUM group, one copy."""
                    xt_sb = xt_box[0]
                    pv2t = psm.tile([128, 2, 192], F32, tag="sm", name="pv2")
                    for jj in range(2):
                        j = jp * 2 + jj
                        for ec in range(ECH):
                            nc.tensor.matmul(
                                pv2t[:, jj, :],
                                xt_sb[:, ec, j * 128:(j + 1) * 128],
                                wv_sb[:, ec, :],
                                start=(jj == 0 and ec == 0),
                                stop=(jj == 1 and ec == ECH - 1),
                                skip_group_check=True,
                            )
                    kc0 = st * 4 + jp * 2
                    pv3 = pv2t.rearrange("p a (h d) -> p a h d", d=64)
                    v8 = v_sb[:, kc0:kc0 + 2, :, 0:64]
                    cp8(v8, pv3)
                    nc.vector.tensor_sub(vr_sb[:, kc0:kc0 + 2, :, :], pv3, v8)

                rcs = (0, 1, 2) if st < QST else (1, 2)
                pieces = [dma]
                pieces += [(lambda rc=rc: qk(rc)) for rc in rcs]
                if o["vp2"]:
                    pieces += [(lambda jp=jp: vp2(jp)) for jp in range(2)]
                else:
                    pieces += [(lambda j=j: vp(j)) for j in range(4)]
                return pieces

            def phase1(st):
                for p in phase1_pieces(st):
                    p()

            # attention unit: one (key-group g, head h, query-block qb)
            kcgs = [(g * KCGo, min(KCGo, KC - g * KCGo))
                    for g in range((KC + KCGo - 1) // KCGo)]

            started_pairs = set()
            exp_ctr = [0]
            n_groups_total = len(kcgs) * HPC * QST

            def exp_engine():
                i = exp_ctr[0]
                exp_ctr[0] += 1
                f = o["dve_g"]
                return "D" if (i * f) // n_groups_total != \
                    ((i + 1) * f) // n_groups_total else "A"

            def attn_unit(gs, h, qb):
                """Several key-groups of one (h, qb) into one PSUM
                accumulator (128-wide slots for DoubleRow alignment)."""
                po = psm.tile([128, QST, 128], F32, tag="sm")
                nc.tensor.matmul(po[:, 0, 0:1], zcol, zrow[:, 0:1],
                                 start=True, stop=False,
                                 skip_group_check=True)
                for gi, g in enumerate(gs):
                    kc0, klen = kcgs[g]
                    ps = psum.tile([128, KCGo, 512], F32, tag="big")
                    for j in range(klen):
                        kc = kc0 + j
                        if QK8 and (kc % 2 == 1):
                            nc.tensor.matmul(
                                ps[:, j, :], kt8_h(h, kc), qt8_h(h, qb),
                                start=True, stop=True, perf_mode=DR,
                            )
                        else:
                            nc.tensor.matmul(
                                ps[:, j, :], kt_h(h, kc), qt_h(h, qb),
                                start=True, stop=True,
                            )
                    pt = pts.tile([128, KCGo, 512], F8, tag="pt")
                    if exp_engine() == "A":
                        nc.scalar.activation(
                            pt[:, 0:klen, :], ps[:, 0:klen, :],
                            mybir.ActivationFunctionType.Exp, scale=0.125,
                        )
                    else:
                        nc.vector.tensor_scalar(
                            pt[:, 0:klen, :].bitcast(I8), ps[:, 0:klen, :],
                            EXP_A, EXP_B, op0=MUL, op1=ADD,
                        )
                    last_u = gi == len(gs) - 1
                    # P@[V8|1] + P@Vr8 as fp8 DoubleRow over kc pairs
                    for qt in range(QST):
                        qc = qt * 128
                        for jj in range(0, klen, 2):
                            last = (last_u and qt == QST - 1 and
                                    jj == klen - 2)
                            nc.tensor.matmul(
                                po[:, qt, 0:65],
                                pt[:, jj:jj + 2, qc:qc + 128],
                                v_sb[:, kc0 + jj:kc0 + jj + 2, h, :],
                                start=False, stop=False,
                                perf_mode=DR, skip_group_check=True,
                            )
                            nc.tensor.matmul(
                                po[:, qt, 0:64],
                                pt[:, jj:jj + 2, qc:qc + 128],
                                vr_sb[:, kc0 + jj:kc0 + jj + 2, h, :],
                                start=False, stop=last,
                                perf_mode=DR, skip_group_check=True,
                            )
                acc = po_acc[:, h * QST + qb, :, :]
                pov = po[:, :, 0:65]
                if (h, qb) not in started_pairs:
                    started_pairs.add((h, qb))
                    nc.vector.tensor_copy(acc, pov)
                else:
                    nc.vector.tensor_add(acc, acc, pov)

            def normalize(h, qb):
                for qt in range(QST):
                    acc = po_acc[:, h * QST + qb, qt, :]
                    rec = small.tile([128, 1], F32, tag="rec")
                    nc.vector.reciprocal(rec, acc[:, 64:65])
                    a_sb = small.tile([128, 64], BF16, tag="a")
                    nc.gpsimd.tensor_scalar(
                        a_sb, acc[:, 0:64], rec, None, op0=MUL)
                    ptr = (psm if o["tr_tag"] == "sm" else psum).tile(
                        [64, 128], BF16, tag=o["tr_tag"])
                    nc.tensor.transpose(ptr, a_sb, ident)
                    qcol = qb * 512 + qt * 128
                    dst = (atA[h * 64:(h + 1) * 64, qcol:qcol + 128]
                           if h < 2 else atB[0:64, qcol:qcol + 128])
                    if o["at_eng"] == "A":
                        nc.scalar.copy(dst, ptr)
                    else:
                        nc.vector.tensor_copy(dst, ptr)

            def _op_half(q, pf, n0, n1):
                nc.tensor.matmul(pf[:, n0:n1],
                                 atA[:, q * 128:(q + 1) * 128],
                                 woA[:, n0:n1], start=True, stop=False)
                nc.tensor.matmul(pf[:, n0:n1],
                                 atB[:, q * 128:(q + 1) * 128],
                                 woB[:, n0:n1], start=False, stop=True)

            def outproj_q(q):
                # needs atA/atB complete for all heads at these query columns
                pf3 = psum.tile([128, KCGo, 512], F32, tag="big")
                pf = pf3.rearrange("p a b -> p (a b)")
                _op_half(q, pf, 0, 512)
                _op_half(q, pf, 512, 768)
                ob = outs.tile([128, E], F32, tag="ob")
                nc.vector.tensor_copy(ob, pf[:, 0:E])
                nc.sync.dma_start(out=out[q * 128:(q + 1) * 128, :], in_=ob)

            # ---- software-pipelined emission ----
            # unit (g, h, qb) is ready once seq-tile max(st(g), qb) is
            # projected; groups of one (h, qb) landing in the same slot are
            # merged (up to cap) to share an accumulator bank + DVE add.
            raw_at = {s: [] for s in range(ST)}
            for g, (kc0, klen) in enumerate(kcgs):
                st_g = (kc0 + klen - 1) // 4
                for qb in range(QST):
                    for h in range(HPC):
                        s_eff = max(st_g, qb)
                        if o["slot2"]:
                            s_eff = min(ST - 1, s_eff | 1)
                        raw_at[s_eff].append((g, h, qb))
            units_at = {}
            for s in range(ST):
                merged = {}
                order = []
                for (g, h, qb) in raw_at[s]:
                    if (h, qb) not in merged:
                        merged[(h, qb)] = []
                        order.append((h, qb))
                    merged[(h, qb)].append(g)
                units = []
                cap = max(1, o["cap_kc"] // KCGo)
                for (h, qb) in order:
                    gs = sorted(merged[(h, qb)])
                    for i in range(0, len(gs), cap):
                        units.append((gs[i:i + cap], h, qb))
                units_at[s] = units
            if o["unit_order"] == "qb":
                for s in range(ST):
                    units_at[s].sort(key=lambda u: (u[2], u[1]))
            elif o["unit_order"] == "h":
                for s in range(ST):
                    units_at[s].sort(key=lambda u: (u[1], u[2]))
            last_s = ST - 1
            # final batch: qb-major so each qb's normalize+outproj fuses in
            units_at[last_s].sort(key=lambda u: (u[2], u[1]))

            def emit_batch(s, units):
                """Interleave next seq-tile's projection pieces among units."""
                if o["interleave"] and s + 1 < ST:
                    pieces = phase1_pieces(s + 1)
                else:
                    pieces = []
                work = []
                if o["piece_order"] == "front":
                    work += [("p", p) for p in pieces]
                    work += [("u", u) for u in units]
                else:
                    n = max(len(units), 1)
                    per = len(pieces) / n
                    acc_p = 0.0
                    pi = 0
                    for i, u in enumerate(units):
                        work.append(("u", u))
                        acc_p += per
                        while pi < len(pieces) and acc_p >= pi + 1:
                            work.append(("p", pieces[pi]))
                            pi += 1
                    while pi < len(pieces):
                        work.append(("p", pieces[pi]))
                        pi += 1
                # In the last batch, a qb's normalize/outproj tail is deferred
                # and interleaved with the NEXT qb's units, so ACT keeps
                # running exps while PE does the tail matmuls.
                remaining = {}
                if s == last_s:
                    for (gs, h, qb) in units:
                        remaining[qb] = remaining.get(qb, 0) + 1
                pending_tail = []

                def tail_pieces(qb):
                    ps = [(lambda hh=hh, qb=qb: normalize(hh, qb))
                          for hh in range(HPC)]
                    ps += [(lambda qq=qq, qb=qb: outproj_q(qb * 4 + qq))
                           for qq in range(4)]
                    return ps

                for kind, item in work:
                    if kind == "u":
                        gs, h, qb = item
                        attn_unit(gs, h, qb)
                        for _ in range(o["tail_pops"]):
                            if pending_tail:
                                pending_tail.pop(0)()
                        if s == last_s:
                            remaining[qb] -= 1
                            if remaining[qb] == 0:
                                pending_tail += tail_pieces(qb)
                    else:
                        item()
                for p in pending_tail:
                    p()

            if o["interleave"]:
                phase1(0)
                for s in range(ST):
                    emit_batch(s, units_at[s])
            else:
                for s in range(ST):
                    phase1(s)
                for s in range(ST):
                    emit_batch(s, units_at[s])

    nc.compile()
    _NC_CACHE[key] = nc
    return nc


def make_in_maps(x, w_qkv, w_out, b_out):
    """Shard full inputs into 8 per-core input maps."""
    x = np.asarray(x, dtype=np.float32)
    w_qkv = np.asarray(w_qkv, dtype=np.float32)
    w_out = np.asarray(w_out, dtype=np.float32)
    b_out = np.asarray(b_out, dtype=np.float32)
    xt = np.ascontiguousarray(x.reshape(S, E).T).astype(
        ml_dtypes.bfloat16)                               # [E, S]
    xt_roll = np.ascontiguousarray(
        np.concatenate([xt[:, QL:], xt[:, :QL]], axis=1))  # for seq-half 1
    in_maps = []
    for c in range(N_CORES):
        hg, sh = c // 2, c % 2
        q_rows = w_qkv[hg * DL:(hg + 1) * DL]             # [192, 768]
        k_rows = w_qkv[E + hg * DL:E + (hg + 1) * DL]
        v_rows = w_qkv[2 * E + hg * DL:2 * E + (hg + 1) * DL]
        wqk_in = np.ascontiguousarray(
            np.concatenate([q_rows, k_rows], axis=0).T).astype(
                ml_dtypes.bfloat16)                       # [768, 384]
        wv_in = np.ascontiguousarray(v_rows.T).astype(ml_dtypes.bfloat16)
        wo_in = np.zeros((DL + 1, E), np.float32)
        wo_in[:DL] = w_out[:, hg * DL:(hg + 1) * DL].T    # [192, 768]
        if hg == 0:
            wo_in[DL] = b_out
        in_maps.append({
            "onesrow": np.ones((1, QL), ml_dtypes.bfloat16),
            "xt": xt if sh == 0 else xt_roll,
            "wqk": wqk_in,
            "wv": wv_in,
            "wo": np.ascontiguousarray(wo_in).astype(ml_dtypes.bfloat16),
        })
    return in_maps


def gather_out(results):
    """Sum head-group partials per seq-half, concat halves -> [1, S, E]."""
    halves = []
    for sh in range(2):
        acc = np.zeros((QL, E), np.float64)
        for hg in range(4):
            acc += results[hg * 2 + sh]["out"]
        halves.append(acc.astype(np.float32))
    return np.concatenate(halves, axis=0)[None]


def kernel(x, w_qkv, w_out, b_out):
    nc = build_nc()
    in_maps = make_in_maps(x, w_qkv, w_out, b_out)
    res = run_bass_kernel_spmd(nc, in_maps, core_ids=list(range(N_CORES)))
    return gather_out(res.results)
